# revision 92
# baseline (speedup 1.0000x reference)
"""AutoregressiveMlpMixer forward on 8 Trainium2 NeuronCores (Bass/Tile).

Strategy
- Pure data parallelism: 64 batch items -> 8 per core, weights replicated.
- The reverse cumsum over tokens is folded into tok_w1 on the host; LN2 /
  final-LN affine params are folded into the following matmul weights.
- ALL 8 items are processed per channel-MLP weight pass, so channel
  weights stream from HBM once per block (4x less weight traffic than a
  2-item grouping). All matmul operands are f32r: neuronxcc rejects mixed
  f32r/bf16 matmuls, and a bf16 moving operand forces an InstLdweights
  split per matmul that saturates the PE sequencer.
- The inter-block state X lives in SBUF in NORMAL [token, channel] layout
  (16 tiles of [128, 768]): LN1 stats read X directly (no transposes, no
  PSUM round-trip), and the head needs no transposes either. Only the
  post-LN2 activations are transposed (PE) into a c-major Zt tile for the
  channel-MLP contraction.
- Channel-MLP runs in fp8e4m3 DoubleRow matmuls (two 128-deep K-planes
  per instruction at 0.5 PE cycles/output element = 4x f32r) with full
  error compensation: every operand T is split host/device-side into
  q(T) + r(T) (both fp8, r = T - dequant(q(T)) under RNE), and each
  contraction evaluates q*q + q*r + r*q, dropping only the O(2^-8) r*r
  term. Net PE cost is 0.75x of f32r with ~0.1% relative error per
  matmul. Weights are host-quantized at 64x scale to clear fp8's
  subnormal floor; the 64x propagates into the block output X and is
  absorbed by LayerNorm's scale invariance (only E's gelu applies a
  1/64 ACT scale to keep h at true scale for its own quantization).
- Channel-MLP: E (1st matmul+gelu) produces h[mt] per 512-col group,
  gelu lands in a f32 staging tile, then DVE copies the fp8 q(h) and
  GPSIMD computes the fp8 residual; F (2nd matmul) accumulates
  mt-PAIR-chunks of kchunk=3 pairs (6 mt) into [128, 768] PSUM tiles
  (each matmul slice stays within one 2KB bank) with h as the
  stationary operand, then folds into X via single 768-wide DVE adds
  (ACT copy on round 0). GPSIMD cannot access PSUM on TRN2, so all
  PSUM-reading elementwise work runs on DVE/ACT.
- Per-item work is a software pipeline (A: LN1 stats, Y+B: LN1 apply +
  first token matmul, D: LN2 finish/apply + transpose + Zt quantize, C:
  second token matmul + LN2 stats) emitted as A(i+2), Y/B(i), D(i-2),
  C(i): D's PE transposes cover the B->C gelu-drain bubble, A runs two
  items ahead, and D two behind. The last two D stages and the next
  block's A stages/weight DMAs interleave into chunk-0/last-chunk fold
  rounds of the channel MLP, so the PE's in-order queue never waits on
  the serial LN chains.
- E and F are software-pipelined by one column group inside each chunk:
  F(cg) consumes exactly the qh/rh columns E(cg) produced, so running F
  one cg behind keeps the ACT->DVE->GPSIMD quantize chain off the PE
  critical path.
- Elementwise work is spread across all four non-PE engines by measured
  balance: LN2 apply (zn) runs on GPSIMD, q(Zt) and the residuals on
  DVE, LN applies/gelus/PSUM copies on ACT. GPSIMD cannot read PSUM, so
  PSUM-sourced ops stay on DVE/ACT.
- rsqrt is a Newton chain on DVE; LN applies are single ACT ops
  out = Identity(rstd*x + (-mu*rstd)) using per-partition AP scale/bias.
  The ACT engine only ever needs the gelu table (Copy/Identity/Gelu share
  one act-func set) -> no act-table reloads.
- ch_b2 (when nonzero) is applied as a rank-1 ones x b2row matmul into
  the F accumulator; zero biases skip it entirely.
"""

import sys

sys.path.insert(0, "/opt/trn_rl_repo")

import numpy as np

import concourse.bass as bass
import concourse.tile as tile
from concourse import bacc, masks, mybir

f32 = mybir.dt.float32
f32r = mybir.dt.float32r
bf16 = mybir.dt.bfloat16
fp8 = mybir.dt.float8e4
AF = mybir.ActivationFunctionType
ALU = mybir.AluOpType
DR = mybir.MatmulPerfMode.DoubleRow
WS = 64.0   # fp8 weight pre-scale (keeps 0.02-std weights out of subnormals)

# Model dims (hardcoded per problem spec)
B, CIN, H, W = 64, 2, 32, 32
N = 256          # tokens
C = 768          # hidden dim
TOK = 512        # tokens_mlp_dim
CH = 3072        # channels_mlp_dim
L = 8            # blocks
K = 2048         # classes
EPS = 1e-5

NCORES = 8
IPC = B // NCORES    # items per core = 8
NT = N // 128        # 2 token tiles per item
CT = C // 128        # 6 channel tiles
MT = CH // 128       # 24 channel-mlp tiles
TT = TOK // 128      # 4 token-mlp tiles
CC = (512, 256)      # channel free-dim chunks for 768
CCO = (0, 512)
CW = 512             # E/F column-group width (2 items)


def _ln_finish(nc, pool, st, magic_t, mode="dve", sfx=""):
    """bn_aggr + Newton rsqrt on DVE. st: [128, s, 6] bn_stats.
    Returns (nb, rstd) column APs with nb = -(mu*rstd), so the LN apply is
    a single ACT op: out = Identity(rstd*x + nb)."""
    i32 = mybir.dt.int32
    mv = pool.tile([128, 2], f32, tag="ln_mv" + sfx, bufs=8, name="mv")
    nc.vector.bn_aggr(out=mv, in_=st)
    v = mv[:, 1:2]
    eng = nc.vector
    eng.tensor_scalar_add(v, v, float(EPS))
    iv = pool.tile([128, 1], i32, tag="rs_i" + sfx, bufs=8, name="iv")
    eng.tensor_scalar(iv, v.bitcast(i32), 1, None,
                      ALU.logical_shift_right)
    eng.tensor_tensor(iv, magic_t[0], iv, ALU.subtract)
    y = iv.bitcast(f32)
    t = pool.tile([128, 1], f32, tag="rs_t" + sfx, bufs=8, name="t")
    # 2 Newton iterations: error ~4e-6 after the magic seed (~3.4%), well
    # below the f32r matmul noise floor; a 3rd iteration is pure overhead
    for _ in range(2):
        eng.tensor_mul(t, y, y)
        eng.tensor_mul(t, t, v)
        eng.tensor_scalar(t, t, -0.5, 1.5, ALU.mult, ALU.add)
        eng.tensor_mul(y, y, t)
    nb = pool.tile([128, 1], f32, tag="ln_nb" + sfx, bufs=8, name="nb")
    eng.tensor_scalar(nb, mv[:, 0:1], y, -1.0, ALU.mult, ALU.mult)
    return nb, y


def _ln_stats(nc, pool, x, magic_t, mode="dve", sfx="", stats_eng=None):
    """LN stats of x[128, C] over the free dim. Returns (nb, rstd) cols."""
    st = pool.tile([128, 2, 6], f32, tag="ln_st" + sfx, bufs=8, name="st")
    eng = stats_eng or nc.vector
    for ci, (cw, co) in enumerate(zip(CC, CCO)):
        eng.bn_stats(out=st[:, ci, :], in_=x[:, co:co + cw])
    return _ln_finish(nc, pool, st, magic_t, mode, sfx)


def build(items=IPC, blocks=L, has_g1=False, has_b1=False,
          has_chb2=False, kchunk=3):
    """Build the SPMD program for one core processing `items` batch items."""
    nc = bacc.Bacc("TRN2", target_bir_lowering=False, debug=False)
    nw = items * N
    ncg = (nw + CW - 1) // CW

    # ---- DRAM tensors (names = in_map keys) ----
    pt = nc.dram_tensor("pt", [9, nw], f32r, kind="ExternalInput")
    wq = nc.dram_tensor("wq", [9, C], f32r, kind="ExternalInput")
    bl = max(blocks, 1)
    tokw1c = nc.dram_tensor("tokw1c", [bl, NT, 128, TOK], f32r, kind="ExternalInput")
    tokw2 = nc.dram_tensor("tokw2", [bl, TT, 128, N], f32r, kind="ExternalInput")
    tokb1 = nc.dram_tensor("tokb1", [bl, 128, TT], f32, kind="ExternalInput")
    qw1g = nc.dram_tensor("qw1g", [bl, MT, 128, CT * 128], fp8, kind="ExternalInput")
    rw1g = nc.dram_tensor("rw1g", [bl, MT, 128, CT * 128], fp8, kind="ExternalInput")
    vb1 = nc.dram_tensor("vb1", [bl, 128, MT], f32, kind="ExternalInput")
    qw2p = nc.dram_tensor("qw2p", [bl, MT // 2, 128, 2, C], fp8, kind="ExternalInput")
    rw2p = nc.dram_tensor("rw2p", [bl, MT // 2, 128, 2, C], fp8, kind="ExternalInput")
    chb2r = nc.dram_tensor("chb2r", [bl, C], f32r, kind="ExternalInput")
    headwg = nc.dram_tensor("headwg", [CT, 128, K], f32r, kind="ExternalInput")
    headb = nc.dram_tensor("headb", [1, K], f32r, kind="ExternalInput")
    ln1g = nc.dram_tensor("ln1g", [bl, C], f32, kind="ExternalInput")
    ln1b = nc.dram_tensor("ln1b", [bl, C], f32, kind="ExternalInput")
    out = nc.dram_tensor("out", [items, K], f32, kind="ExternalOutput")

    with tile.TileContext(nc) as tc:
        with tc.tile_pool(name="const", bufs=1) as const, \
             tc.tile_pool(name="xstate", bufs=1) as xstate:
            magic_i = const.tile([128, 1], mybir.dt.int32, name="magic_i")
            nc.vector.memset(magic_i, 0x5F3759DF)
            eps_col = const.tile([128, 1], f32, name="eps_col")
            nc.vector.memset(eps_col, EPS)
            magic_t = (magic_i, eps_col)
            ident = const.tile([128, 128], f32, name="ident")
            masks.make_identity(nc, ident)
            identr = const.tile([128, 128], f32r, name="identr")
            nc.vector.tensor_copy(identr, ident)

            # persistent state, NORMAL layout: X[i][t] = [128(tok), C]
            X = [[xstate.tile([128, C], f32r, tag=f"x_{i}_{t}",
                              name=f"x_{i}_{t}") for t in range(NT)]
                 for i in range(items)]
            ones1_f = const.tile([1, 128], f32, name="ones1_f")
            nc.vector.memset(ones1_f, 1.0)
            ones1 = const.tile([1, 128], f32r, name="ones1")
            nc.vector.tensor_copy(ones1, ones1_f)
            # post-LN2 state consumed by the channel MLP, c-major fp8
            # (quantized + residual); slice [:, kc, :] is channel-tile kc
            qZt_all = xstate.tile([128, CT * nw], fp8, tag="qzt", name="qzt")
            qZtv = qZt_all.rearrange("p (k n) -> p k n", k=CT)
            rZt_all = xstate.tile([128, CT * nw], fp8, tag="rzt", name="rzt")
            rZtv = rZt_all.rearrange("p (k n) -> p k n", k=CT)

            # ---------------- stem (writes X transposed) ----------------
            with tc.tile_pool(name="stem", bufs=1) as stem, \
                 tc.tile_pool(name="ps_stem", bufs=4, space="PSUM") as ps_stem:
                ptt = stem.tile([9, nw], f32r)
                nc.sync.dma_start(out=ptt, in_=pt[:, :])
                wqt = stem.tile([9, C], f32r)
                nc.sync.dma_start(out=wqt, in_=wq[:, :])
                for i in range(items):
                    for t in range(NT):
                        o = i * N + t * 128
                        pss = ps_stem.tile([128, C], f32,
                                           tag="pss", bufs=4, name="pss")
                        for cw, co in zip(CC, CCO):
                            nc.tensor.matmul(pss[:, co:co + cw],
                                             ptt[:, o:o + 128],
                                             wqt[:, co:co + cw],
                                             start=True, stop=True)
                        nc.scalar.activation(out=X[i][t], in_=pss,
                                             func=AF.Copy)

            # ---------------- mixer blocks ----------------
            with tc.tile_pool(name="tokw", bufs=2) as tokwp, \
                 tc.tile_pool(name="lnp", bufs=4) as lnp, \
                 tc.tile_pool(name="acts", bufs=1) as acts, \
                 tc.tile_pool(name="wstream", bufs=3) as wstream, \
                 tc.tile_pool(name="ps_mm", bufs=4, space="PSUM") as ps_mm:

                blk_w = {}

                def emit_tok_weights(l):
                    w = {}
                    w1c_t = tokwp.tile([128, NT, TOK], f32r, tag="w1c",
                                       name="w1c")
                    nc.sync.dma_start(out=w1c_t,
                                      in_=tokw1c[l].rearrange("k p t -> p k t"))
                    w2_t = tokwp.tile([128, TT, N], f32r, tag="w2", name="w2")
                    nc.sync.dma_start(out=w2_t,
                                      in_=tokw2[l].rearrange("k p n -> p k n"))
                    b1_t = tokwp.tile([128, TT], f32, tag="b1", name="b1")
                    nc.sync.dma_start(out=b1_t, in_=tokb1[l])
                    vb1_t = tokwp.tile([128, MT], f32, tag="vb1", name="vb1")
                    nc.sync.dma_start(out=vb1_t, in_=vb1[l])
                    w.update(w1c=w1c_t, w2=w2_t, b1=b1_t, vb1=vb1_t)
                    if has_chb2:
                        chb2_t = tokwp.tile([1, C], f32r, tag="chb2",
                                            name="chb2")
                        nc.sync.dma_start(out=chb2_t, in_=chb2r[l:l + 1])
                        w["chb2"] = chb2_t
                    if has_g1:
                        g1_t = tokwp.tile([128, C], f32, tag="g1", name="g1")
                        nc.sync.dma_start(
                            out=g1_t,
                            in_=ln1g.ap()[l:l + 1, :].partition_broadcast(128))
                        w["g1"] = g1_t
                    if has_b1:
                        b1v_t = tokwp.tile([128, C], f32, tag="b1v", name="b1v")
                        nc.sync.dma_start(
                            out=b1v_t,
                            in_=ln1b.ap()[l:l + 1, :].partition_broadcast(128))
                        w["b1v"] = b1v_t
                    return w

                def emit_A(l, i):
                    """LN1 stats straight from the token-major X tiles.
                    rsqrt chain on Pool (tag sfx "1")."""
                    mus = [_ln_stats(nc, lnp, X[i][t], magic_t, "dve", "1")
                           for t in range(NT)]
                    return (mus,)

                def emit_Y(l, i, mus):
                    """LN1 apply for item i (ACT; fills the ACT queue while
                    the previous item's B matmuls run on PE)."""
                    w = blk_w[l]
                    Y = []
                    for t in range(NT):
                        nb, rstd = mus[t]
                        yt = lnp.tile([128, C], f32r, tag="y", bufs=4,
                                      name="yt")
                        nc.scalar.activation(
                            out=yt, in_=X[i][t],
                            func=AF.Identity, bias=nb, scale=rstd)
                        if has_g1:
                            nc.vector.tensor_mul(yt, yt, w["g1"])
                        if has_b1:
                            nc.vector.tensor_add(yt, yt, w["b1v"])
                        Y.append(yt)
                    return Y

                def emit_B(l, i, Y, mm_ring=False):
                    """First token-MLP matmul + gelu. mm_ring routes the
                    PSUM tiles through the (EF-idle) mm ring: used for the
                    first items of a block, whose psf slots would otherwise
                    wait on the previous block's fold drain."""
                    w = blk_w[l]
                    w1c_t, w2_t, b1_t = w["w1c"], w["w2"], w["b1"]
                    # ---- B: y1 = gelu(w1cum^T @ Y + b1) ----
                    y1 = []
                    for mt in range(TT):
                        yg = lnp.tile([128, C], f32r, tag="y1g", bufs=6,
                                      name="yg")
                        if mm_ring:
                            for cw, co in zip(CC, CCO):
                                pbp = ps_mm.tile([128, 512], f32, tag="mm",
                                                 bufs=2, name="pbp")
                                for k in range(NT):
                                    nc.tensor.matmul(
                                        pbp[:, :cw],
                                        w1c_t[:, k, mt * 128:(mt + 1) * 128],
                                        Y[k][:, co:co + cw],
                                        start=(k == 0), stop=(k == NT - 1))
                                nc.scalar.activation(
                                    out=yg[:, co:co + cw], in_=pbp[:, :cw],
                                    func=AF.Gelu, bias=b1_t[:, mt:mt + 1],
                                    scale=1.0)
                        else:
                            pb = ps_mm.tile([128, C], f32, tag="psf", bufs=3,
                                            name="pb")
                            for cw, co in zip(CC, CCO):
                                for k in range(NT):
                                    nc.tensor.matmul(
                                        pb[:, co:co + cw],
                                        w1c_t[:, k, mt * 128:(mt + 1) * 128],
                                        Y[k][:, co:co + cw],
                                        start=(k == 0), stop=(k == NT - 1))
                            nc.scalar.activation(
                                out=yg, in_=pb, func=AF.Gelu,
                                bias=b1_t[:, mt:mt + 1], scale=1.0)
                        y1.append(yg)
                    return y1

                def emit_C(l, i, y1, mm_ring=False):
                    """Second token-MLP matmul + LN2 stats from PSUM."""
                    w = blk_w[l]
                    w2_t = w["w2"]
                    cd = []
                    for t in range(NT):
                        y2t = lnp.tile([128, C], f32, tag="y2", bufs=6,
                                       name="y2t")
                        st = lnp.tile([128, 2, 6], f32, tag="ln_st2",
                                      bufs=8, name="st")
                        pc = ps_mm.tile([128, C], f32, tag="psf", bufs=3,
                                        name="pc")
                        for cw, co in zip(CC, CCO):
                            for k in range(TT):
                                nc.tensor.matmul(
                                    pc[:, co:co + cw],
                                    w2_t[:, k, t * 128:(t + 1) * 128],
                                    y1[k][:, co:co + cw],
                                    start=(k == 0), stop=(k == TT - 1))
                        if t == 0:
                            nc.scalar.activation(out=y2t, in_=pc,
                                                 func=AF.Copy)
                        else:
                            nc.vector.tensor_copy(y2t, pc)
                        for ci, (cw, co) in enumerate(zip(CC, CCO)):
                            nc.vector.bn_stats(out=st[:, ci, :],
                                               in_=pc[:, co:co + cw])
                        cd.append((y2t, st))
                    return cd

                def emit_D(l, i, cd):
                    """LN2 finish+apply + transpose into Zt columns, then
                    fp8 quantize (ACT) + residual (DVE) from PSUM."""
                    for t in range(NT):
                        y2t, st = cd[t]
                        nb, rstd = _ln_finish(nc, lnp, st, magic_t, "dve",
                                              "2")
                        zn = lnp.tile([128, C], f32r, tag="z", bufs=3,
                                      name="zn")
                        nc.gpsimd.tensor_scalar(
                            out=zn, in0=y2t, scalar1=rstd, scalar2=nb,
                            op0=ALU.mult, op1=ALU.add)
                        ptr = ps_mm.tile([128, C], f32r, tag="psf",
                                         bufs=3, name="ptrT")
                        for cc in range(CT):
                            nc.tensor.transpose(
                                ptr[:, cc * 128:(cc + 1) * 128],
                                zn[:, cc * 128:(cc + 1) * 128],
                                identr)
                        o = i * N + t * 128
                        ptrv = ptr.rearrange("p (k q) -> p k q", k=CT)
                        if t == 0:
                            nc.scalar.activation(
                                out=qZtv[:, :, o:o + 128], in_=ptrv,
                                func=AF.Copy)
                        else:
                            nc.vector.tensor_copy(qZtv[:, :, o:o + 128],
                                                  ptrv)
                        nc.vector.tensor_tensor(
                            rZtv[:, :, o:o + 128], ptrv,
                            qZtv[:, :, o:o + 128], ALU.subtract)

                npt = MT // 2                       # 12 mt-pairs
                nchunk = (npt + kchunk - 1) // kchunk

                def emit_chunk_ws(l, ci):
                    """DMA one chunk's fp8 weights; returns (hs, p0, pn)."""
                    hs = []
                    p0 = ci * kchunk
                    pn = min(kchunk, npt - p0)
                    for jp in range(p0, p0 + pn):
                        qw1_t = wstream.tile([128, 2, CT, 128], fp8,
                                             tag="qw1",
                                             bufs=kchunk, name="qw1_t")
                        nc.sync.dma_start(
                            out=qw1_t,
                            in_=qw1g[l, 2 * jp:2 * jp + 2]
                            .rearrange("k p (c m) -> p k c m", c=CT))
                        rw1_t = wstream.tile([128, 2, CT, 128], fp8,
                                             tag="rw1",
                                             bufs=kchunk, name="rw1_t")
                        nc.sync.dma_start(
                            out=rw1_t,
                            in_=rw1g[l, 2 * jp:2 * jp + 2]
                            .rearrange("k p (c m) -> p k c m", c=CT))
                        qw2_t = wstream.tile([128, 2, C], fp8, tag="qw2",
                                             bufs=kchunk, name="qw2_t")
                        nc.sync.dma_start(out=qw2_t, in_=qw2p[l, jp])
                        rw2_t = wstream.tile([128, 2, C], fp8, tag="rw2",
                                             bufs=kchunk, name="rw2_t")
                        nc.sync.dma_start(out=rw2_t, in_=rw2p[l, jp])
                        qh = acts.tile([128, 2, nw], fp8, tag="qh",
                                       bufs=kchunk, name="qh")
                        rh = acts.tile([128, 2, nw], fp8, tag="rh",
                                       bufs=kchunk, name="rh")
                        hs.append((qh, rh, qw1_t, rw1_t, qw2_t, rw2_t, jp))
                    return hs, p0, pn

                def emit_E_round(l, hs, cg):
                    """E for one column group (items 2cg, 2cg+1): fp8
                    DoubleRow matmuls + gelu into f32 staging + qh (DVE)
                    + rh residual (GPSIMD)."""
                    vb1_t = blk_w[l]["vb1"]
                    co = cg * CW
                    cw = min(CW, nw - co)
                    for (qh, rh, qw1_t, rw1_t, _, _, jp) in hs:
                        for p in range(2):
                            mt = 2 * jp + p
                            pe = ps_mm.tile([128, 512], f32, tag="mm",
                                            bufs=2, name="pe")
                            egrp = [(qw1_t, qZtv), (qw1_t, rZtv),
                                    (rw1_t, qZtv)]
                            ne = len(egrp) * (CT // 2)
                            ei = 0
                            for (wt, zt) in egrp:
                                for kc in range(CT // 2):
                                    nc.tensor.matmul(
                                        pe[:, :cw],
                                        wt[:, p, 2 * kc:2 * kc + 2, :],
                                        zt[:, 2 * kc:2 * kc + 2, co:co + cw],
                                        start=(ei == 0), stop=(ei == ne - 1),
                                        perf_mode=DR)
                                    ei += 1
                            hf = lnp.tile([128, 512], f32, tag="hf",
                                          bufs=5, name="hf")
                            nc.scalar.activation(
                                out=hf[:, :cw], in_=pe[:, :cw],
                                func=AF.Gelu, bias=vb1_t[:, mt:mt + 1],
                                scale=1.0 / WS)
                            nc.vector.tensor_copy(
                                qh[:, p, co:co + cw], hf[:, :cw])
                            nc.gpsimd.tensor_tensor(
                                rh[:, p, co:co + cw], hf[:, :cw],
                                qh[:, p, co:co + cw], ALU.subtract)

                def emit_F_round(l, hs, fg, p0, pn):
                    """F fold round for items 2fg, 2fg+1 over chunk hs."""
                    chb2_t = blk_w[l].get("chb2")
                    for i2 in (2 * fg, 2 * fg + 1):
                        if i2 >= items:
                            continue
                        for t in range(NT):
                            o = i2 * N + t * 128
                            pf = ps_mm.tile([128, C], f32, tag="psf",
                                            bufs=3, name="pf")
                            for fw, fo in zip(CC, CCO):
                                bias_mm = (p0 == 0 and chb2_t is not None)
                                if bias_mm:
                                    # channel bias: rank-1 ones x b2
                                    nc.tensor.matmul(
                                        pf[:, fo:fo + fw], ones1,
                                        chb2_t[:, fo:fo + fw],
                                        start=True, stop=False)
                                nf = 3 * pn
                                fi = 0
                                for gsel in range(3):
                                    for (qh, rh, _, _, qw2_t, rw2_t,
                                         _jp) in hs:
                                        hsel = rh if gsel == 2 else qh
                                        wsel = (rw2_t if gsel == 1
                                                else qw2_t)
                                        nc.tensor.matmul(
                                            pf[:, fo:fo + fw],
                                            hsel[:, :, o:o + 128],
                                            wsel[:, :, fo:fo + fw],
                                            start=(fi == 0 and not bias_mm),
                                            stop=(fi == nf - 1),
                                            perf_mode=DR)
                                        fi += 1
                            if p0 == 0:
                                nc.scalar.activation(
                                    out=X[i2][t], in_=pf, func=AF.Copy)
                            else:
                                nc.vector.tensor_add(
                                    X[i2][t], X[i2][t], pf)

                def emit_EF_all(l, a_hook=None, d_queue=None,
                                defer_last=False):
                    """All chunks, E/F software-pipelined by one column
                    group so the quantize chain stays off the PE critical
                    path. d_queue: deferred D-stages consumed in chunk 0's
                    early fold rounds. a_hook(fg) fires after the LAST
                    chunk's fold of column group fg (its items' X final).
                    defer_last: skip the very last F round and return it
                    as (hs, p0, pn, fg) so the caller can inject it into
                    the next block's item-0 slot, covering the B(0)->C(0)
                    gelu-drain bubble there with ready PE work."""
                    ret = None
                    for ci in range(nchunk):
                        hs, p0, pn = emit_chunk_ws(l, ci)
                        last = ci == nchunk - 1
                        for cg in range(ncg + 1):
                            if cg < ncg:
                                emit_E_round(l, hs, cg)
                            if cg == 0:
                                continue
                            fg = cg - 1
                            if last and fg == ncg - 1 and defer_last:
                                ret = [(hs, p0, pn, fg)]
                                continue
                            emit_F_round(l, hs, fg, p0, pn)
                            if ci == 0 and d_queue:
                                emit_D(l, *d_queue.pop(0))
                            if last and a_hook is not None:
                                a_hook(fg)
                    return ret

                pre = {}
                pending_F = None
                for l in range(blocks):
                    if l not in blk_w:
                        blk_w[l] = emit_tok_weights(l)
                    for i in range(min(2, items)):
                        if i not in pre:
                            pre[i] = emit_A(l, i)
                    cd = {}
                    for i in range(items):
                        if i + 2 < items:
                            pre[i + 2] = emit_A(l, i + 2)
                        # Y(i)+B(i) -> D(i-2) -> C(i): D's PE transposes
                        # (inputs always ready by now) cover the gelu-drain
                        # bubble between B's last matmul and C's first
                        ring = False
                        y1 = None
                        if i not in cd:
                            y1 = emit_B(l, i, emit_Y(l, i, *pre.pop(i)),
                                        mm_ring=ring)
                        if i <= 1 and pending_F:
                            hs_p, p0_p, pn_p, fg_p = pending_F.pop(0)
                            emit_F_round(l - 1, hs_p, fg_p, p0_p, pn_p)
                        if i - 2 >= 0:
                            emit_D(l, i - 2, cd.pop(i - 2))
                        if y1 is not None:
                            cd[i] = emit_C(l, i, y1, mm_ring=ring)
                    # last two D-stages are deferred into chunk 0's fold
                    # rounds (E only needs their Zt columns at cg 3)
                    d_queue = [(i, cd.pop(i)) for i in (items - 2, items - 1)
                               if i in cd]
                    if l + 1 < blocks:
                        def a_hook(cg, nl=l + 1):
                            # next block's A/BC interleave into the last
                            # chunk's fold rounds: X[l+1] columns finalize
                            # per column group
                            if cg == 0:
                                blk_w[nl] = emit_tok_weights(nl)
                                for i2 in range(min(2, items)):
                                    pre[i2] = emit_A(nl, i2)
                    else:
                        a_hook = None
                    pending_F = emit_EF_all(l, a_hook, d_queue,
                                            defer_last=(l + 1 < blocks))


            # ---------------- final LN + token-mean + head ----------------
            with tc.tile_pool(name="headp", bufs=1) as headp, \
                 tc.tile_pool(name="lnf", bufs=4) as lnf, \
                 tc.tile_pool(name="ps_h", bufs=2, space="PSUM") as ps_h:
                invn_f = headp.tile([128, 2], f32)
                nc.vector.memset(invn_f, 1.0 / N)
                invn_col = headp.tile([128, 2], f32r)
                nc.vector.tensor_copy(invn_col, invn_f)
                ones8_f = headp.tile([1, items], f32)
                nc.vector.memset(ones8_f, 1.0)
                ones8 = headp.tile([1, items], f32r)
                nc.vector.tensor_copy(ones8, ones8_f)
                xmall = headp.tile([128, CT, items], f32r)
                for i in range(items):
                    xh = []
                    for t in range(NT):
                        nb, rstd = _ln_stats(nc, lnf, X[i][t], magic_t,
                                             "dve", "2")
                        xht = lnf.tile([128, C], f32r, tag="xh", bufs=4,
                                       name="xht")
                        if t == 0:
                            nc.scalar.activation(
                                out=xht, in_=X[i][t],
                                func=AF.Identity, bias=nb, scale=rstd)
                        else:
                            nc.gpsimd.tensor_scalar(
                                out=xht, in0=X[i][t], scalar1=rstd,
                                scalar2=nb, op0=ALU.mult, op1=ALU.add)
                        xh.append(xht)
                    for ct in range(CT):
                        pxm = ps_h.tile([128, 2], f32, tag="pxm", bufs=4, name="pxm")
                        for t in range(NT):
                            nc.tensor.matmul(pxm,
                                             xh[t][:, ct * 128:(ct + 1) * 128],
                                             invn_col,
                                             start=(t == 0), stop=(t == NT - 1))
                        nc.scalar.activation(out=xmall[:, ct, i:i + 1],
                                             in_=pxm[:, 0:1], func=AF.Copy)
                hb_t = headp.tile([1, K], f32r)
                nc.sync.dma_start(out=hb_t, in_=headb[:, :])
                outsb = headp.tile([items, K], f32)
                hw_ts = {}
                for jc in range(K // 512):
                    for ct in range(CT):
                        hw_t = headp.tile([128, 512], f32r, tag="hw",
                                          bufs=24, name="hw_t")
                        nc.sync.dma_start(
                            out=hw_t,
                            in_=headwg[ct, :, jc * 512:(jc + 1) * 512])
                        hw_ts[(jc, ct)] = hw_t
                for jc in range(K // 512):
                    ph = ps_h.tile([items, 512], f32, tag="ph", name="ph")
                    for ct in range(CT):
                        nc.tensor.matmul(ph, xmall[:, ct, :items],
                                         hw_ts[(jc, ct)],
                                         start=(ct == 0), stop=False)
                    nc.tensor.matmul(ph, ones8, hb_t[:, jc * 512:(jc + 1) * 512],
                                     start=False, stop=True)
                    nc.scalar.activation(out=outsb[:, jc * 512:(jc + 1) * 512],
                                         in_=ph, func=AF.Copy)
                nc.sync.dma_start(out=out[:, :], in_=outsb)

    nc.compile()
    return nc


# ---------------------------------------------------------------------------
# host-side preprocessing
# ---------------------------------------------------------------------------

def prep_inputs(inputs, stem_w, stem_b, ln1_g, ln1_b, tok_w1, tok_b1, tok_w2,
                tok_b2, ln2_g, ln2_b, ch_w1, ch_b1, ch_w2, ch_b2, lnf_g, lnf_b,
                head_w, head_b, items=IPC, blocks=L):
    """Returns (shared_map, per_core_list, flags)."""
    import ml_dtypes
    f = np.float32
    b16 = ml_dtypes.bfloat16
    inputs = np.asarray(inputs, f)
    # patches: (B, CIN, 16, 2, 16, 2) -> (B, n=256, q=8); +ones row -> (B,9,256)
    x = inputs.reshape(B, CIN, H // 2, 2, W // 2, 2).transpose(0, 2, 4, 1, 3, 5)
    x = x.reshape(B, N, CIN * 4)
    ptA = np.concatenate([x.transpose(0, 2, 1),
                          np.ones((B, 1, N), f)], axis=1)  # (B, 9, 256)

    wq = np.concatenate([np.asarray(stem_w, f).reshape(C, 8).T,
                         np.asarray(stem_b, f)[None, :]], axis=0)  # (9, C)

    blocks = max(blocks, 1)
    w1cum = np.cumsum(np.asarray(tok_w1, f), axis=1)[:blocks]        # (L, N, TOK)
    tokw1c = np.ascontiguousarray(
        w1cum.reshape(blocks, NT, 128, TOK))
    tokw2 = np.ascontiguousarray(np.asarray(tok_w2, f)[:blocks]
                                 .reshape(blocks, TT, 128, N))
    tokb1 = np.ascontiguousarray(np.asarray(tok_b1, f)[:blocks]
                                 .reshape(blocks, TT, 128).transpose(0, 2, 1))

    g2 = np.asarray(ln2_g, f)[:blocks]
    b2 = np.asarray(ln2_b, f)[:blocks]
    cw1 = np.asarray(ch_w1, f)[:blocks]
    fp8n = ml_dtypes.float8_e4m3
    w1g_full = np.float32(WS) * g2[:, :, None] * cw1                  # (L, C, CH)
    w1g64 = np.ascontiguousarray(
        w1g_full.reshape(blocks, CT, 128, MT, 128)
        .transpose(0, 3, 2, 1, 4)).reshape(blocks, MT, 128, CT * 128)
    qw1g = w1g64.astype(fp8n)
    rw1g = (w1g64 - qw1g.astype(f)).astype(fp8n)
    v = np.einsum("lc,lcm->lm", b2, cw1) + np.asarray(ch_b1, f)[:blocks]
    vb1 = np.ascontiguousarray(v.reshape(blocks, MT, 128).transpose(0, 2, 1))
    w2_64 = np.float32(WS) * np.asarray(ch_w2, f)[:blocks]            # (L, CH, C)
    w2p = np.ascontiguousarray(
        w2_64.reshape(blocks, MT // 2, 2, 128, C).transpose(0, 1, 3, 2, 4))
    qw2p = w2p.astype(fp8n)
    rw2p = (w2p - qw2p.astype(f)).astype(fp8n)
    chb2r = np.ascontiguousarray(np.float32(WS)
                                 * np.asarray(ch_b2, f)[:blocks])

    gf = np.asarray(lnf_g, f)
    bf = np.asarray(lnf_b, f)
    hw = np.asarray(head_w, f)
    headwg = np.ascontiguousarray(
        (gf[:, None] * hw).reshape(CT, 128, K))
    headb = (bf @ hw + np.asarray(head_b, f)).reshape(1, K).astype(f)

    ln1g = np.ascontiguousarray(np.asarray(ln1_g, f)[:blocks])
    ln1b = np.ascontiguousarray(np.asarray(ln1_b, f)[:blocks])
    has_g1 = not np.all(ln1g == 1.0)
    has_b1 = not np.all(ln1b == 0.0)
    has_chb2 = not np.all(chb2r == 0.0)

    shared = dict(wq=wq, tokw1c=tokw1c, tokw2=tokw2, tokb1=tokb1, qw1g=qw1g,
                  rw1g=rw1g, vb1=vb1, qw2p=qw2p, rw2p=rw2p, chb2r=chb2r,
                  headwg=headwg, headb=headb, ln1g=ln1g, ln1b=ln1b)

    per_core = []
    for c in range(NCORES):
        sel = ptA[c * IPC:(c + 1) * IPC][:items]  # (items, 9, 256)
        ptc = np.ascontiguousarray(sel.transpose(1, 0, 2).reshape(9, items * N))
        per_core.append(dict(pt=ptc))
    return shared, per_core, dict(has_g1=has_g1, has_b1=has_b1,
                              has_chb2=has_chb2)


_CACHE = {}


def kernel(**inputs):
    from concourse.bass_utils import run_bass_kernel_spmd
    shared, per_core, flags = prep_inputs(**inputs)
    key = (flags["has_g1"], flags["has_b1"], flags["has_chb2"])
    if key not in _CACHE:
        _CACHE[key] = build(has_g1=flags["has_g1"], has_b1=flags["has_b1"],
                            has_chb2=flags["has_chb2"])
    nc = _CACHE[key]
    in_maps = [{**shared, **pc} for pc in per_core]
    res = run_bass_kernel_spmd(nc, in_maps, core_ids=list(range(NCORES)))
    outs = [r["out"] for r in res.results]
    return np.concatenate(outs, axis=0).astype(np.float32)

